# revision 1
# baseline (speedup 1.0000x reference)
"""Hyperbolic (Poincare ball, c=1) bilinear 2x upsample.

Math: the geodesic midpoint of x, y on the Poincare ball reduces exactly
to mid = P*x + Q*y, with per-pixel scalars P, Q functions of the three
channel dot products (|x|^2, |y|^2, <x,y>).  The reference's `a`/`b`
tensors are slices of mid_h and the cell centers are vertical geodesic
midpoints of mid_h, so only 3 midpoint passes are needed.

Sharding: pure data parallel over batch (B=8 -> one image per worker).
The Bass/Tile kernel for this op hit a toolchain codegen limit ("too
many sync wait commands" on every Tile-generated NEFF, including
trivial ones) and the XLA-on-neuron path ICEs in the tensorizer, so
this fallback computes with numpy.
"""

import numpy as np

B, C, H, W = 8, 64, 128, 128


def _PQ(x2, y2, xy):
    g = 1.0 - 2.0 * xy
    be = 1.0 - x2
    D1 = g + x2 * y2
    r1 = 1.0 / D1
    a1 = (g + y2) * r1
    b1 = be * r1
    w2 = a1 * a1 * x2 + b1 * b1 * y2 - 2.0 * a1 * b1 * xy
    s = np.sqrt(np.maximum(1.0 - w2, 1e-30))
    u = 1.0 / (1.0 + s)
    xs = u * (b1 * xy - a1 * x2)
    s2 = u * u * w2
    h = 1.0 + 2.0 * xs
    r2 = 1.0 / (h + x2 * s2)
    p = (h + s2) * r2
    q = be * u * r2
    return p - q * a1, q * b1


def kernel(x: np.ndarray) -> np.ndarray:
    from concurrent.futures import ThreadPoolExecutor

    x = np.ascontiguousarray(x, dtype=np.float32)
    out = np.empty((B, C, 2 * H, 2 * W), np.float32)
    with ThreadPoolExecutor(max_workers=B) as ex:
        list(ex.map(lambda b: _one(x[b : b + 1], out[b : b + 1]), range(B)))
    return out


def _one(x: np.ndarray, out: np.ndarray) -> None:

    S = np.sum(x * x, axis=1, keepdims=True, dtype=np.float32)
    Hh = np.sum(x[:, :, :, : W - 1] * x[:, :, :, 1:], axis=1, keepdims=True, dtype=np.float32)
    Vv = np.sum(x[:, :, : H - 1, :] * x[:, :, 1:, :], axis=1, keepdims=True, dtype=np.float32)

    Ph, Qh = _PQ(S[:, :, :, : W - 1], S[:, :, :, 1:], Hh)
    mh = Ph * x[:, :, :, : W - 1] + Qh * x[:, :, :, 1:]

    Pv, Qv = _PQ(S[:, :, : H - 1, :], S[:, :, 1:, :], Vv)
    mv = Pv * x[:, :, : H - 1, :] + Qv * x[:, :, 1:, :]

    Smh = np.sum(mh * mh, axis=1, keepdims=True, dtype=np.float32)
    Vmh = np.sum(mh[:, :, : H - 1, :] * mh[:, :, 1:, :], axis=1, keepdims=True, dtype=np.float32)
    Pc, Qc = _PQ(Smh[:, :, : H - 1, :], Smh[:, :, 1:, :], Vmh)
    ctr = Pc * mh[:, :, : H - 1, :] + Qc * mh[:, :, 1:, :]

    out[:, :, 0::2, 0::2] = x
    out[:, :, 0::2, 1 : 2 * (W - 1) : 2] = mh
    out[:, :, 1 : 2 * (H - 1) : 2, 0::2] = mv
    out[:, :, 1 : 2 * (H - 1) : 2, 1 : 2 * (W - 1) : 2] = ctr
    out[:, :, :, -1] = out[:, :, :, -2]
    out[:, :, -1, :] = out[:, :, -2, :]


if __name__ == "__main__":
    xv = np.load("/tmp/x_full.npy")
    got = kernel(xv)
    exp = np.load("/tmp/expected.npy")
    print("norm rel err:", np.linalg.norm((got - exp).ravel()) / np.linalg.norm(exp.ravel()))



# revision 3
# speedup vs baseline: 13.5216x; 13.5216x over previous
"""Hyperbolic (Poincare ball, c=1) bilinear 2x upsample — Bass/Tile kernel
running data-parallel on 8 Trainium2 NeuronCores (one image per core).

Math: the geodesic midpoint of x, y on the Poincare ball reduces exactly to
mid = P*x + Q*y, with per-pixel scalars P, Q functions of the three channel
dot products (|x|^2, |y|^2, <x,y>).  The reference's cell centers are
vertical geodesic midpoints of the horizontal midpoints, so three midpoint
passes suffice.

Device pipeline per core (image (64, 128, 128) -> (64, 256, 256)):
 1. DMA the image in channel-major layout [64(c), H*W].
 2. PE transposes to pixel-major [128(w), H, C] (plus a copy shifted one
    pixel right for the horizontal pairs).
 3. Channel dot products via DVE elementwise multiply + segmented reduce;
    |x_{w+1}|^2 via a PE shift-matmul.
 4. P,Q coefficient fields on [128, H] tiles (~28 small DVE/ACT ops).
 5. Midpoint combines as broadcast-AP tensor_tensor ops.
 6. Output rows: PE transposes (two rows per matmul) into PSUM, interleaving
    PSUM->SBUF copies on ACT+DVE, contiguous DMA stores.

The TileContext output is post-processed by `_split_multi_waits` because the
walrus build in this container accepts only ONE sync wait per instruction.

The compiled executable is cached in a module global: the first kernel()
call compiles (~2 min), subsequent calls only pay data movement + execute.
"""

import numpy as np

B, C, H, W = 8, 64, 128, 128
HO, WO = 2 * H, 2 * W
N_CORES = 8

_STATE = None


def _split_multi_waits(nc):
    import concourse.mybir as mybir

    n_split = 0
    for f in nc.m.functions:
        for blk in f.blocks:
            insts = blk.instructions
            out = []
            changed = False
            for inst in insts:
                si = inst.sync_info
                if si is not None and len(si.on_wait) > 1:
                    waits = list(si.on_wait)
                    for k, w in enumerate(waits[:-1]):
                        nop = mybir.InstNoOp(
                            name=f"{inst.name}__wsplit{k}",
                            engine=inst.engine,
                            sync_info=mybir.SyncInfo(on_wait=[w], on_update=[]),
                            bass_nofuse=True,
                        )
                        out.append(nop)
                        n_split += 1
                    inst.sync_info = mybir.SyncInfo(
                        on_wait=[waits[-1]], on_update=list(si.on_update)
                    )
                    changed = True
                out.append(inst)
            if changed:
                blk.instructions = out
    return n_split


def _make_consts() -> np.ndarray:
    cst = np.zeros((128, 256), np.float32)
    cst[:, 0:128] = np.eye(128, dtype=np.float32)
    sh = np.zeros((128, 128), np.float32)
    for m in range(127):
        sh[m + 1, m] = 1.0
    sh[127, 127] = 1.0  # last column pairs with itself -> midpoint(x,x)=x
    cst[:, 128:256] = sh
    return cst


def _build_nc():
    import concourse.bass as bass
    import concourse.mybir as mybir
    from concourse.tile import TileContext

    F32 = mybir.dt.float32
    MULT = mybir.AluOpType.mult
    ADD = mybir.AluOpType.add
    SUB = mybir.AluOpType.subtract
    AXX = mybir.AxisListType.X
    PAD = 128

    nc = bass.Bass()
    x = nc.declare_dram_parameter("x", [C, H * W], F32, isOutput=False)
    cst_d = nc.declare_dram_parameter("cst", [128, 256], F32, isOutput=False)
    y = nc.declare_dram_parameter("y", [C, HO, WO], F32, isOutput=True)

    with TileContext(nc) as tc:
        with (
            tc.tile_pool(name="big", bufs=1) as big,
            tc.tile_pool(name="xp", bufs=1) as xp,
            tc.tile_pool(name="xs", bufs=1) as xs,
            tc.tile_pool(name="t1", bufs=1) as t1p,
            tc.tile_pool(name="sm", bufs=1) as sm,
            tc.tile_pool(name="pqt", bufs=10) as pqt,
            tc.tile_pool(name="chk", bufs=2) as chk,
            tc.tile_pool(name="orow", bufs=3) as orow,
            tc.tile_pool(name="cstp", bufs=1) as cstp,
            tc.tile_pool(name="pio", bufs=6, space="PSUM") as pio,
            tc.tile_pool(name="pps", bufs=1, space="PSUM") as pps,
        ):
            cst = cstp.tile([128, 256], F32)
            nc.sync.dma_start(out=cst[:], in_=cst_d[:])
            id128 = cst[:, 0:128]
            id64 = cst[0:64, 0:64]
            shiftm = cst[:, 128:256]

            chan = big.tile([C, H * W + PAD], F32, tag="big")
            nc.sync.dma_start(out=chan[:, 0 : H * W], in_=x[:])
            nc.gpsimd.memset(chan[:, H * W : H * W + PAD], 0.0)

            Xpix = xp.tile([128, H, C], F32)
            Xsh = xs.tile([128, H, C], F32)

            for g in range(16):  # 8 rows per group
                pX = pio.tile([128, 8, C], F32, tag="pio", name=f"pX{g}")
                pS = pio.tile([128, 8, C], F32, tag="pio", name=f"pS{g}")
                for r in range(8):
                    h = 8 * g + r
                    nc.tensor.matmul(
                        pX[:, r, :], chan[:, h * W : (h + 1) * W], id64,
                        is_transpose=True, start=True, stop=True,
                    )
                    nc.tensor.matmul(
                        pS[:, r, :], chan[:, h * W + 1 : (h + 1) * W + 1], id64,
                        is_transpose=True, start=True, stop=True,
                    )
                nc.scalar.copy(Xpix[:, 8 * g : 8 * g + 8, :], pX[:])
                nc.scalar.copy(Xsh[:, 8 * g : 8 * g + 8, :], pS[:])
            # patch: Xsh[127] := Xpix[127] (last col pairs with itself)
            nc.sync.dma_start(out=Xsh[127:128, :, :], in_=Xpix[127:128, :, :])

            T1 = t1p.tile([128, H, C], F32)

            Sx = sm.tile([128, H], F32, tag="Sx")
            Sxsh = sm.tile([128, H], F32, tag="Sxsh")
            xyh = sm.tile([128, H], F32, tag="xyh")
            xyv = sm.tile([128, H], F32, tag="xyv")
            Smh = sm.tile([128, H], F32, tag="Smh")
            xyc = sm.tile([128, H], F32, tag="xyc")

            nc.scalar.square(T1[:], Xpix[:])
            nc.vector.tensor_reduce(out=Sx[:], in_=T1[:], axis=AXX, op=ADD)
            pSh = pps.tile([128, H], F32)
            nc.tensor.matmul(pSh[:], shiftm, Sx[:], start=True, stop=True)
            nc.scalar.copy(Sxsh[:], pSh[:])

            nc.vector.tensor_tensor(out=T1[:], in0=Xpix[:], in1=Xsh[:], op=MULT)
            nc.vector.tensor_reduce(out=xyh[:], in_=T1[:], axis=AXX, op=ADD)

            nc.vector.tensor_tensor(
                out=T1[:, 0 : H - 1, :], in0=Xpix[:, 0 : H - 1, :],
                in1=Xpix[:, 1:H, :], op=MULT,
            )
            nc.vector.tensor_reduce(
                out=xyv[:, 0 : H - 1], in_=T1[:, 0 : H - 1, :], axis=AXX, op=ADD
            )

            _pqn = [0]

            def emit_pq(x2, y2, xy, P, Q, L):
                def t():
                    _pqn[0] += 1
                    tl = pqt.tile([128, L], F32, tag="pqt", name=f"pqt{_pqn[0]}")
                    return tl[:]

                g = t(); be = t(); r1 = t(); a1 = t(); b1 = t()
                tmp = t(); tmp2 = t(); w2 = t(); u = t(); s2 = t()
                tt = nc.vector.tensor_tensor
                nc.vector.tensor_scalar(out=g, in0=xy, scalar1=-2.0, scalar2=1.0, op0=MULT, op1=ADD)
                nc.vector.tensor_scalar(out=be, in0=x2, scalar1=-1.0, scalar2=1.0, op0=MULT, op1=ADD)
                tt(out=tmp, in0=x2, in1=y2, op=MULT)
                tt(out=tmp, in0=g, in1=tmp, op=ADD)        # D1
                nc.vector.reciprocal(out=r1, in_=tmp)
                tt(out=tmp, in0=g, in1=y2, op=ADD)
                tt(out=a1, in0=tmp, in1=r1, op=MULT)
                tt(out=b1, in0=be, in1=r1, op=MULT)
                tt(out=tmp, in0=a1, in1=x2, op=MULT)       # a1*x2 (reused for xs)
                tt(out=w2, in0=tmp, in1=a1, op=MULT)
                tt(out=tmp2, in0=b1, in1=y2, op=MULT)
                tt(out=tmp2, in0=tmp2, in1=b1, op=MULT)
                tt(out=w2, in0=w2, in1=tmp2, op=ADD)
                tt(out=tmp2, in0=a1, in1=b1, op=MULT)
                tt(out=tmp2, in0=tmp2, in1=xy, op=MULT)
                nc.vector.tensor_scalar(out=tmp2, in0=tmp2, scalar1=-2.0, scalar2=None, op0=MULT)
                tt(out=w2, in0=w2, in1=tmp2, op=ADD)       # w2
                nc.vector.tensor_scalar(out=tmp2, in0=w2, scalar1=-1.0, scalar2=1.0, op0=MULT, op1=ADD)
                nc.vector.tensor_scalar_max(out=tmp2, in0=tmp2, scalar1=1e-20)
                nc.scalar.sqrt(tmp2, tmp2)
                nc.vector.tensor_scalar(out=tmp2, in0=tmp2, scalar1=1.0, scalar2=None, op0=ADD)
                nc.vector.reciprocal(out=u, in_=tmp2)      # u = 1/(1+s)
                tt(out=tmp2, in0=b1, in1=xy, op=MULT)
                tt(out=tmp2, in0=tmp2, in1=tmp, op=SUB)
                tt(out=tmp2, in0=tmp2, in1=u, op=MULT)     # xs
                nc.vector.tensor_scalar(out=tmp2, in0=tmp2, scalar1=2.0, scalar2=1.0, op0=MULT, op1=ADD)  # h2
                tt(out=s2, in0=u, in1=u, op=MULT)
                tt(out=s2, in0=s2, in1=w2, op=MULT)        # s2
                tt(out=tmp, in0=x2, in1=s2, op=MULT)
                tt(out=tmp, in0=tmp, in1=tmp2, op=ADD)
                nc.vector.reciprocal(out=tmp, in_=tmp)     # r2
                tt(out=w2, in0=tmp2, in1=s2, op=ADD)
                tt(out=w2, in0=w2, in1=tmp, op=MULT)       # (h2+s2)*r2
                tt(out=u, in0=be, in1=u, op=MULT)
                tt(out=u, in0=u, in1=tmp, op=MULT)         # q_ = be*u*r2
                tt(out=Q, in0=u, in1=b1, op=MULT)
                tt(out=tmp, in0=u, in1=a1, op=MULT)
                tt(out=P, in0=w2, in1=tmp, op=SUB)

            Ph = sm.tile([128, H], F32, tag="Ph")
            Qh = sm.tile([128, H], F32, tag="Qh")
            Pv = sm.tile([128, H - 1], F32, tag="Pv")
            Qv = sm.tile([128, H - 1], F32, tag="Qv")
            Pc = sm.tile([128, H - 1], F32, tag="Pc")
            Qc = sm.tile([128, H - 1], F32, tag="Qc")

            emit_pq(Sx[:], Sxsh[:], xyh[:], Ph[:], Qh[:], H)
            emit_pq(Sx[:, 0 : H - 1], Sx[:, 1:H], xyv[:, 0 : H - 1], Pv[:], Qv[:], H - 1)

            mh = big.tile([128, H, C], F32, tag="big")
            PhB = Ph[:, :, None].broadcast_to([128, H, C])
            QhB = Qh[:, :, None].broadcast_to([128, H, C])
            nc.vector.tensor_tensor(out=T1[:], in0=Xpix[:], in1=PhB, op=MULT)
            nc.vector.tensor_tensor(out=mh[:], in0=Xsh[:], in1=QhB, op=MULT)
            nc.vector.tensor_tensor(out=mh[:], in0=mh[:], in1=T1[:], op=ADD)

            nc.scalar.square(T1[:], mh[:])
            nc.vector.tensor_reduce(out=Smh[:], in_=T1[:], axis=AXX, op=ADD)
            nc.vector.tensor_tensor(
                out=T1[:, 0 : H - 1, :], in0=mh[:, 0 : H - 1, :],
                in1=mh[:, 1:H, :], op=MULT,
            )
            nc.vector.tensor_reduce(
                out=xyc[:, 0 : H - 1], in_=T1[:, 0 : H - 1, :], axis=AXX, op=ADD
            )
            emit_pq(Smh[:, 0 : H - 1], Smh[:, 1:H], xyc[:, 0 : H - 1], Pc[:], Qc[:], H - 1)

            # Output transposes stack TWO rows per matmul: out [128, 128] with
            # partitions 0:64 = row h channels, 64:128 = row h+1 channels.
            CH = 16
            for ci in range(H // CH):
                c0, c1 = ci * CH, (ci + 1) * CH
                cm = min(c1, H - 1)
                L = cm - c0
                mv = chk.tile([128, CH, C], F32, tag="mv")
                ct = chk.tile([128, CH, C], F32, tag="ct")
                PvB = Pv[:, c0:cm, None].broadcast_to([128, L, C])
                QvB = Qv[:, c0:cm, None].broadcast_to([128, L, C])
                PcB = Pc[:, c0:cm, None].broadcast_to([128, L, C])
                QcB = Qc[:, c0:cm, None].broadcast_to([128, L, C])
                nc.vector.tensor_tensor(
                    out=T1[:, c0:cm, :], in0=Xpix[:, c0:cm, :], in1=PvB, op=MULT)
                nc.vector.tensor_tensor(
                    out=mv[:, 0:L, :], in0=Xpix[:, c0 + 1 : cm + 1, :], in1=QvB, op=MULT)
                nc.vector.tensor_tensor(
                    out=mv[:, 0:L, :], in0=mv[:, 0:L, :], in1=T1[:, c0:cm, :], op=ADD)
                nc.vector.tensor_tensor(
                    out=T1[:, c0:cm, :], in0=mh[:, c0:cm, :], in1=PcB, op=MULT)
                nc.vector.tensor_tensor(
                    out=ct[:, 0:L, :], in0=mh[:, c0 + 1 : cm + 1, :], in1=QcB, op=MULT)
                nc.vector.tensor_tensor(
                    out=ct[:, 0:L, :], in0=ct[:, 0:L, :], in1=T1[:, c0:cm, :], op=ADD)
                if L < CH:
                    nc.gpsimd.memset(mv[:, L:CH, :], 0.0)
                    nc.gpsimd.memset(ct[:, L:CH, :], 0.0)

                G2 = 4  # row-pairs per psum group; pairs (h, h+1), h even
                for gi in range(CH // (2 * G2)):
                    ppx = pio.tile([128, G2, 128], F32, tag="pio", name=f"ppx{ci}_{gi}")
                    ppm = pio.tile([128, G2, 128], F32, tag="pio", name=f"ppm{ci}_{gi}")
                    ppv = pio.tile([128, G2, 128], F32, tag="pio", name=f"ppv{ci}_{gi}")
                    ppc = pio.tile([128, G2, 128], F32, tag="pio", name=f"ppc{ci}_{gi}")
                    for p in range(G2):
                        h = c0 + gi * 2 * G2 + 2 * p
                        r = h - c0
                        nc.tensor.matmul(
                            ppx[:, p, :], Xpix[:, h : h + 2, :], id128,
                            is_transpose=True, start=True, stop=True)
                        nc.tensor.matmul(
                            ppm[:, p, :], mh[:, h : h + 2, :], id128,
                            is_transpose=True, start=True, stop=True)
                        nc.tensor.matmul(
                            ppv[:, p, :], mv[:, r : r + 2, :], id128,
                            is_transpose=True, start=True, stop=True)
                        nc.tensor.matmul(
                            ppc[:, p, :], ct[:, r : r + 2, :], id128,
                            is_transpose=True, start=True, stop=True)
                    ste = orow.tile([128, G2, 256], F32, tag="orow", name=f"ste{ci}_{gi}")
                    sto = orow.tile([128, G2, 256], F32, tag="orow", name=f"sto{ci}_{gi}")
                    nc.scalar.copy(ste[:, :, 0:256:2], ppx[:])
                    nc.vector.tensor_copy(ste[:, :, 1:256:2], ppm[:])
                    nc.scalar.copy(sto[:, :, 0:256:2], ppv[:])
                    nc.vector.tensor_copy(sto[:, :, 1:256:2], ppc[:])
                    h0 = c0 + gi * 2 * G2
                    last = h0 + 2 * G2 >= H
                    nc.sync.dma_start(
                        out=y[:, 2 * h0 : 2 * h0 + 4 * (G2 - 1) + 1 : 4, :],
                        in_=ste[0:64, :, :])
                    nc.sync.dma_start(
                        out=y[:, 2 * h0 + 2 : 2 * h0 + 2 + 4 * (G2 - 1) + 1 : 4, :],
                        in_=ste[64:128, :, :])
                    nc.sync.dma_start(
                        out=y[:, 2 * h0 + 1 : 2 * h0 + 1 + 4 * (G2 - 1) + 1 : 4, :],
                        in_=sto[0:64, :, :])
                    if not last:
                        nc.sync.dma_start(
                            out=y[:, 2 * h0 + 3 : 2 * h0 + 3 + 4 * (G2 - 1) + 1 : 4, :],
                            in_=sto[64:128, :, :])
                    else:
                        nc.sync.dma_start(
                            out=y[:, 2 * h0 + 3 : 2 * h0 + 3 + 4 * (G2 - 2) + 1 : 4, :],
                            in_=sto[64:128, 0 : G2 - 1, :])
                        # row 255 := row 254
                        nc.sync.dma_start(out=y[:, 255, :], in_=ste[64:128, G2 - 1, :])

    _split_multi_waits(nc)
    return nc


def _build_state():
    """Compile once and cache a jitted shard_map callable over 8 cores."""
    import jax
    from jax.sharding import Mesh, PartitionSpec
    from jax.experimental.shard_map import shard_map
    import concourse.mybir as mybir
    from concourse import bass2jax

    nc = _build_nc()
    bass2jax.install_neuronx_cc_hook()

    partition_name = (
        nc.partition_id_tensor.name if nc.partition_id_tensor else None
    )
    in_names = []
    out_names = []
    out_avals = []
    for alloc in nc.m.functions[0].allocations:
        if not isinstance(alloc, mybir.MemoryLocationSet):
            continue
        name = alloc.memorylocations[0].name
        if alloc.kind == "ExternalInput":
            if name != partition_name:
                in_names.append(name)
        elif alloc.kind == "ExternalOutput":
            out_names.append(name)
            out_avals.append(
                jax.core.ShapedArray(
                    tuple(alloc.tensor_shape), mybir.dt.np(alloc.dtype)
                )
            )
    n_params = len(in_names)
    n_outs = len(out_names)
    all_names = list(in_names + out_names)
    if partition_name is not None:
        all_names.append(partition_name)
    all_names = tuple(all_names)

    def _body(*args):
        operands = list(args)
        if partition_name is not None:
            operands.append(bass2jax.partition_id_tensor())
        outs = bass2jax._bass_exec_p.bind(
            *operands,
            out_avals=tuple(out_avals),
            in_names=all_names,
            out_names=tuple(out_names),
            lowering_input_output_aliases=(),
            sim_require_finite=True,
            sim_require_nnan=True,
            nc=nc,
        )
        return tuple(outs)

    devices = jax.devices()[:N_CORES]
    mesh = Mesh(np.asarray(devices), ("core",))
    in_specs = (PartitionSpec("core"),) * (n_params + n_outs)
    out_specs = (PartitionSpec("core"),) * n_outs
    donate = tuple(range(n_params, n_params + n_outs))
    sharded = jax.jit(
        shard_map(_body, mesh=mesh, in_specs=in_specs, out_specs=out_specs,
                  check_rep=False),
        donate_argnums=donate,
        keep_unused=True,
    )

    cst = _make_consts()
    cst_concat = np.concatenate([cst] * N_CORES, axis=0)
    return {
        "sharded": sharded,
        "in_names": in_names,
        "cst_concat": cst_concat,
        "mesh": mesh,
    }


def _get_state():
    global _STATE
    if _STATE is None:
        _STATE = _build_state()
    return _STATE


def kernel(x: np.ndarray) -> np.ndarray:
    st = _get_state()
    x = np.ascontiguousarray(x, dtype=np.float32)
    # per-core input [64, H*W]; concat along axis 0 == plain reshape
    xc = x.reshape(B * C, H * W)
    zeros = np.zeros((B * C, HO, WO), np.float32)
    args = {"x": xc, "cst": st["cst_concat"]}
    ordered = [args[n] for n in st["in_names"]] + [zeros]
    (out,) = st["sharded"](*ordered)
    return np.asarray(out).reshape(B, C, HO, WO)


if __name__ == "__main__":
    xv = np.load("/tmp/x_full.npy")
    got = kernel(xv)
    exp = np.load("/tmp/expected.npy")
    print("norm rel err:", np.linalg.norm((got - exp).ravel()) / np.linalg.norm(exp.ravel()))
